# revision 1
# baseline (speedup 1.0000x reference)
"""Distributed causal MHA for TRN2 (8 NeuronCores), v4.

Core c: batch c//2; 256-row query blocks {even|odd positions} of that
batch (causal balance). Slot s statically needs 4(s+1) key tiles; key
tile jt serves slots >= jt//4, so the score matmul for (head, jt) is ONE
wide MM over all those slots' query columns (N = 256*(4-jt//4) <= 1024
bf16), followed by ONE wide exp on ACT. Only the first 256-col block
(slot jt//4) straddles the diagonal -> in-place (iota >= mstart) * exp
on DVE. AV accumulates per slot into column-packed PSUM banks with a
ones-column in V producing softmax denominators; normalization happens
once at the end (DMA-packed denominator rows -> one DVE reciprocal ->
K=1 broadcast matmuls). bf16 matmuls, fp32 accumulation, max-free
softmax.
"""

import sys

sys.path.insert(0, "/opt/trn_rl_repo")
import numpy as np
import ml_dtypes
import concourse.bass as bass
import concourse.mybir as mybir
import concourse.tile as tile
from concourse.vector_clock import ScopedClock
from concourse.bass_utils import run_bass_kernel_spmd

B, N, DIM = 4, 2048, 1024
HEADS, DH = 16, 64
INNER = HEADS * DH
SCALE = DH ** -0.5
NQ = 1024
CH = 256
NSLOT = 4
F32 = mybir.dt.float32
BF16 = mybir.dt.bfloat16
AF = mybir.ActivationFunctionType
ALU = mybir.AluOpType

LAST_RESULT = None


def _drain_and_barrier_patched(self, tick_clock, wait_clock):
    nop_inst = self.nc.sync.nop(nofuse=True)
    wait_clock.add_sem_waits(nop_inst.ins, ScopedClock({None: tick_clock.global_clock}))
    si = nop_inst.ins.sync_info
    waits = list(si.on_wait or []) if si else []
    if len(waits) > 1:
        nop_inst.ins.sync_info = mybir.SyncInfo(
            on_wait=waits[:1], on_update=list(si.on_update or [])
        )
        for i in range(1, len(waits)):
            extra = self.nc.sync.nop(nofuse=True)
            extra.ins.sync_info = mybir.SyncInfo(on_wait=[waits[i]], on_update=[])
    self.nc.sync.drain()
    self.nc.all_engine_barrier()
    popped = self.nc._tile_sem_poison_stack.pop()
    assert popped is self._sem_poison
    self.nc.clear_and_free_semaphores(list(self.sems.allocated().values()))
    self.nc.all_engine_barrier()


tile.TileContext._drain_and_barrier = _drain_and_barrier_patched


def _split_multi_waits(nc):
    for f in nc.m.functions:
        for bb in f.blocks:
            insts = bb.instructions
            if not any(
                i.sync_info and i.sync_info.on_wait and len(i.sync_info.on_wait) > 1
                for i in insts
            ):
                continue
            new = []
            for inst in insts:
                si = inst.sync_info
                waits = list(si.on_wait) if si and si.on_wait else []
                if len(waits) > 1:
                    for w in waits[:-1]:
                        nop = mybir.InstNoOp(
                            name=nc.get_next_instruction_name(), ins=[], outs=[]
                        )
                        nop.engine = inst.engine
                        nop.sync_info = mybir.SyncInfo(on_wait=[w], on_update=[])
                        new.append(nop)
                    inst.sync_info = mybir.SyncInfo(
                        on_wait=[waits[-1]], on_update=list(si.on_update or [])
                    )
                new.append(inst)
            bb.instructions = new


def build_graph():
    nc = bass.Bass("TRN2", target_bir_lowering=False)

    p_xT = nc.declare_dram_parameter("xT", [DIM, N], BF16, isOutput=False)
    p_xTq = nc.declare_dram_parameter("xTq", [DIM, NQ], BF16, isOutput=False)
    p_wq = nc.declare_dram_parameter("w_q", [DIM, INNER], BF16, isOutput=False)
    p_wkv = nc.declare_dram_parameter("w_kv", [DIM, 2 * INNER], BF16, isOutput=False)
    p_wbo = nc.declare_dram_parameter("wb_out", [INNER + 1, DIM], BF16, isOutput=False)
    p_ms = nc.declare_dram_parameter("mstart", [128, 16], BF16, isOutput=False)
    p_iota = nc.declare_dram_parameter("iota", [128, CH], BF16, isOutput=False)
    p_out = nc.declare_dram_parameter("out", [NQ, DIM], F32, isOutput=True)

    with tile.TileContext(nc) as tc:
        with (
            tc.tile_pool(name="const", bufs=1) as cst,
            tc.tile_pool(name="qt", bufs=1) as qtp,
            tc.tile_pool(name="vsb", bufs=1) as vsp,
            tc.tile_pool(name="ktr", bufs=1) as ktrp,
        ):
            iota = cst.tile([128, CH], BF16, tag="iota", name="iota")
            nc.sync.dma_start(iota[:, :], p_iota[:, :])
            ms = cst.tile([128, 16], BF16, tag="ms", name="ms")
            nc.sync.dma_start(ms[:, :], p_ms[:, :])
            ones64 = cst.tile([1, 64], F32, tag="ones64", name="ones64")
            nc.vector.memset(ones64[:, :], 1.0)
            onesb = cst.tile([1, 1024], BF16, tag="onesb", name="onesb")
            nc.vector.memset(onesb[:, :], 1.0)

            qt = [qtp.tile([128, NQ], BF16, tag=f"qt{i}", name=f"qt{i}") for i in range(8)]
            vsb = [vsp.tile([128, HEADS * (DH + 1)], BF16, tag=f"v{i}", name=f"v{i}") for i in range(16)]
            ktr = [ktrp.tile([128, N], BF16, tag=f"kt{i}", name=f"kt{i}") for i in range(8)]

            # ---------------- P0a: QT = w_q.T @ xTq ----------------
            with (
                tc.tile_pool(name="xtq", bufs=1) as xtqp,
                tc.tile_pool(name="wqp", bufs=1) as wqp,
                tc.tile_pool(name="ps0", bufs=2, space="PSUM") as ps0,
            ):
                xtq = [xtqp.tile([128, NQ], BF16, tag=f"xtq{i}", name=f"xtq{i}") for i in range(8)]
                for i in range(8):
                    nc.sync.dma_start(xtq[i][:, :], p_xTq[i * 128:(i + 1) * 128, :])
                wq = [wqp.tile([128, INNER], BF16, tag=f"wq{i}", name=f"wq{i}") for i in range(8)]
                for i in range(8):
                    nc.sync.dma_start(wq[i][:, :], p_wq[i * 128:(i + 1) * 128, :])
                for ft in range(8):
                    for tc2 in range(2):
                        pq = ps0.tile([128, 512], F32, name="pq")
                        for kt in range(8):
                            nc.tensor.matmul(
                                pq[:, :],
                                wq[kt][:, ft * 128:(ft + 1) * 128],
                                xtq[kt][:, tc2 * 512:(tc2 + 1) * 512],
                                start=(kt == 0),
                                stop=(kt == 7),
                            )
                        nc.vector.tensor_copy(
                            qt[ft][:, tc2 * 512:(tc2 + 1) * 512], pq[:, :]
                        )

            # ---------------- P0b/P0c: KT resident, V token-major ----------------
            with tc.tile_pool(name="xt", bufs=1) as xtp:
                xt = [xtp.tile([128, N], BF16, tag=f"xt{i}", name=f"xt{i}") for i in range(8)]
                for i in range(8):
                    nc.sync.dma_start(xt[i][:, :], p_xT[i * 128:(i + 1) * 128, :])

                with (
                    tc.tile_pool(name="wkp", bufs=3) as wkp,
                    tc.tile_pool(name="ps1", bufs=1, space="PSUM") as ps1,
                ):
                    for ft in range(8):
                        pk = [ps1.tile([128, 512], F32, tag=f"pk{j}", name=f"pk{j}") for j in range(4)]
                        for kt in range(8):
                            wk = wkp.tile([128, 128], BF16, tag="wk", name="wk")
                            nc.sync.dma_start(
                                wk[:, :],
                                p_wkv[kt * 128:(kt + 1) * 128, ft * 128:(ft + 1) * 128],
                            )
                            for tc4 in range(4):
                                nc.tensor.matmul(
                                    pk[tc4][:, :],
                                    wk[:, :],
                                    xt[kt][:, tc4 * 512:(tc4 + 1) * 512],
                                    start=(kt == 0),
                                    stop=(kt == 7),
                                )
                        for tc4 in range(4):
                            nc.vector.tensor_copy(
                                ktr[ft][:, tc4 * 512:(tc4 + 1) * 512], pk[tc4][:, :]
                            )

                with (
                    tc.tile_pool(name="wvp", bufs=3) as wvp,
                    tc.tile_pool(name="ps2", bufs=1, space="PSUM") as ps2,
                ):
                    for tgrp in range(2):
                        for fc in range(2):
                            pv = [ps2.tile([128, 512], F32, tag=f"pv{j}", name=f"pv{j}") for j in range(8)]
                            for kt in range(8):
                                wv = wvp.tile([128, 512], BF16, tag="wv", name="wv")
                                nc.sync.dma_start(
                                    wv[:, :],
                                    p_wkv[
                                        kt * 128:(kt + 1) * 128,
                                        INNER + fc * 512:INNER + (fc + 1) * 512,
                                    ],
                                )
                                for t8 in range(8):
                                    tt = tgrp * 8 + t8
                                    nc.tensor.matmul(
                                        pv[t8][:, :],
                                        xt[kt][:, tt * 128:(tt + 1) * 128],
                                        wv[:, :],
                                        start=(kt == 0),
                                        stop=(kt == 7),
                                    )
                            for t8 in range(8):
                                tt = tgrp * 8 + t8
                                dst = vsb[tt][
                                    :, fc * 8 * 65:(fc * 8 + 8) * 65
                                ].rearrange("p (g d) -> p g d", g=8)[:, :, 0:64]
                                src = pv[t8][:, :].rearrange("p (g d) -> p g d", g=8)
                                nc.vector.tensor_copy(dst, src)
                    for tt in range(16):
                        nc.vector.memset(
                            vsb[tt][:, :].rearrange("p (g d) -> p g d", g=16)[:, :, 64:65],
                            1.0,
                        )

            # ---------------- P1: attention ----------------
            afp = tc.alloc_tile_pool(name="af", bufs=1)
            af = [afp.tile([128, NQ], BF16, tag=f"af{i}", name=f"af{i}") for i in range(8)]
            anum = [afp.tile([128, NQ], F32, tag=f"an{i}", name=f"an{i}") for i in range(8)]
            dens = afp.tile([64, CH], F32, tag="dens", name="dens")
            with (
                tc.tile_pool(name="work", bufs=3) as wkpool,
                tc.tile_pool(name="psS", bufs=2, space="PSUM") as psS,
                tc.tile_pool(name="psA", bufs=1, space="PSUM") as psA,
            ):
                for hp in range(8):
                    h0, h1 = 2 * hp, 2 * hp + 1
                    kth = ktr[hp]
                    qtile = qt[hp]
                    for g in range(2):
                        slo = 2 * g            # slots {slo, slo+1}
                        nv = {
                            (hi, si): psA.tile([65, CH], F32, tag=f"nv{hi}{si}", name=f"nv{hi}{si}")
                            for hi in range(2) for si in range(2)
                        }
                        pend = None  # (jt, smin, need_mask, [rhs tiles per head])
                        for jt in range(8 * g + 8):
                            smin = max(slo, jt // 4)
                            width = (slo + 2 - smin) * CH
                            need_mask = (jt // 4 == smin)
                            rhss = []
                            for hi, off, h in ((0, 0, h0), (1, 64, h1)):
                                stW = psS.tile([128, 512], F32, tag=f"stW{hi}", name=f"stW{hi}")
                                nc.tensor.matmul(
                                    stW[:, 0:width],
                                    kth[off:off + 64, jt * 128:(jt + 1) * 128],
                                    qtile[off:off + 64, smin * CH:smin * CH + width],
                                    start=True,
                                    stop=True,
                                )
                                eW = wkpool.tile([128, 512], BF16, tag=f"eW{hi}", name=f"eW{hi}")
                                nc.scalar.activation(
                                    eW[:, 0:width], stW[:, 0:width], AF.Exp, scale=SCALE
                                )
                                if need_mask:
                                    em = wkpool.tile([128, CH], BF16, tag=f"em{hi}", name=f"em{hi}")
                                    nc.vector.scalar_tensor_tensor(
                                        em[:, :],
                                        iota[:, :],
                                        ms[:, jt:jt + 1],
                                        eW[:, 0:CH],
                                        ALU.is_ge,
                                        ALU.mult,
                                    )
                                else:
                                    em = None
                                rhss.append((em, eW))
                            if pend is not None:
                                pjt, psmin, pmask, prhss = pend
                                for hi, off, h in ((0, 0, h0), (1, 64, h1)):
                                    pem, peW = prhss[hi]
                                    for si2 in range(psmin, slo + 2):
                                        navm = nv[(hi, si2 - slo)]
                                        rhs = (
                                            pem[:, :]
                                            if (pmask and si2 == psmin)
                                            else peW[:, (si2 - psmin) * CH:(si2 - psmin + 1) * CH]
                                        )
                                        nc.tensor.matmul(
                                            navm[:, :],
                                            vsb[pjt][:, h * 65:(h + 1) * 65],
                                            rhs,
                                            start=(pjt == 0),
                                            stop=(pjt == 4 * si2 + 3),
                                        )
                            pend = (jt, smin, need_mask, rhss)
                        # drain last pending AV
                        pjt, psmin, pmask, prhss = pend
                        for hi, off, h in ((0, 0, h0), (1, 64, h1)):
                            pem, peW = prhss[hi]
                            for si2 in range(psmin, slo + 2):
                                navm = nv[(hi, si2 - slo)]
                                rhs = (
                                    pem[:, :]
                                    if (pmask and si2 == psmin)
                                    else peW[:, (si2 - psmin) * CH:(si2 - psmin + 1) * CH]
                                )
                                nc.tensor.matmul(
                                    navm[:, :],
                                    vsb[pjt][:, h * 65:(h + 1) * 65],
                                    rhs,
                                    start=(pjt == 0),
                                    stop=(pjt == 4 * si2 + 3),
                                )
                        for hi, off, h in ((0, 0, h0), (1, 64, h1)):
                            for si in range(2):
                                s2 = slo + si
                                navm = nv[(hi, si)]
                                nc.vector.tensor_copy(
                                    anum[hp][off:off + 64, s2 * CH:(s2 + 1) * CH],
                                    navm[0:64, :],
                                )
                                den0 = wkpool.tile([1, CH], F32, tag="den0", name="den0", bufs=4)
                                nc.vector.tensor_copy(den0[:, :], navm[64:65, :])
                                r = h * 4 + s2
                                nc.sync.dma_start(dens[r:r + 1, :], den0[:, :])

            # ---------------- P2: batched normalization ----------------
            with (
                tc.tile_pool(name="nrm", bufs=1) as nrmp,
                tc.tile_pool(name="psR", bufs=4, space="PSUM") as psR,
            ):
                rd = nrmp.tile([64, CH], F32, tag="rd", name="rd")
                nc.vector.reciprocal(rd[:, :], dens[:, :])
                for h in range(HEADS):
                    off = (h % 2) * 64
                    for s in range(NSLOT):
                        r = h * 4 + s
                        rdr = nrmp.tile([1, CH], F32, tag="rdr", name="rdr", bufs=8)
                        nc.sync.dma_start(rdr[:, :], rd[r:r + 1, :])
                        rb = psR.tile([64, CH], F32, tag="rb", name="rb")
                        nc.tensor.matmul(rb[:, :], ones64[:, :], rdr[:, :], start=True, stop=True)
                        nc.vector.tensor_mul(
                            af[h // 2][off:off + 64, s * CH:(s + 1) * CH],
                            anum[h // 2][off:off + 64, s * CH:(s + 1) * CH],
                            rb[:, :],
                        )

            # ---------------- P3: out-projection ----------------
            with (
                tc.tile_pool(name="wop", bufs=1) as wop,
                tc.tile_pool(name="wbp", bufs=1) as wbp,
                tc.tile_pool(name="ow", bufs=3) as owp,
                tc.tile_pool(name="psO", bufs=4, space="PSUM") as psO,
            ):
                wo = [wop.tile([128, DIM], BF16, tag=f"wo{i}", name=f"wo{i}") for i in range(8)]
                for i in range(8):
                    nc.sync.dma_start(wo[i][:, :], p_wbo[i * 128:(i + 1) * 128, :])
                wbias = wbp.tile([1, DIM], BF16, tag="wbias", name="wbias")
                nc.sync.dma_start(wbias[:, :], p_wbo[INNER:INNER + 1, :])
                for it in range(8):
                    for oc in range(2):
                        po = psO.tile([128, 512], F32, tag="po", name="po")
                        for ft in range(8):
                            nc.tensor.matmul(
                                po[:, :],
                                af[ft][:, it * 128:(it + 1) * 128],
                                wo[ft][:, oc * 512:(oc + 1) * 512],
                                start=(ft == 0),
                                stop=False,
                            )
                        nc.tensor.matmul(
                            po[:, :],
                            onesb[:, it * 128:(it + 1) * 128],
                            wbias[:, oc * 512:(oc + 1) * 512],
                            start=False,
                            stop=True,
                        )
                        ot = owp.tile([128, 512], F32, tag="ot", name="ot")
                        nc.vector.tensor_copy(ot[:, :], po[:, :])
                        nc.sync.dma_start(
                            p_out[it * 128:(it + 1) * 128, oc * 512:(oc + 1) * 512],
                            ot[:, :],
                        )
            afp.release()

    _split_multi_waits(nc)
    return nc


_GRAPH = None


def _get_graph():
    global _GRAPH
    if _GRAPH is None:
        _GRAPH = build_graph()
    return _GRAPH


def _core_row_blocks(c):
    par = c % 2
    return [2 * s + par for s in range(NSLOT)]


def kernel(x, mask, w_qkv, w_out, b_out):
    global LAST_RESULT
    x = np.asarray(x, dtype=np.float32)
    w_qkv = np.asarray(w_qkv, dtype=np.float32)
    w_out = np.asarray(w_out, dtype=np.float32)
    b_out = np.asarray(b_out, dtype=np.float32)

    nc = _get_graph()

    BF = ml_dtypes.bfloat16
    w_q = np.ascontiguousarray(w_qkv[:, :INNER].astype(BF))
    w_kv = np.ascontiguousarray(w_qkv[:, INNER:].astype(BF))
    wb = np.ascontiguousarray(np.vstack([w_out, b_out[None, :]]).astype(BF))
    iota = np.broadcast_to(np.arange(CH, dtype=np.float32), (128, CH)).astype(BF).copy()

    xT = [np.ascontiguousarray(x[b].T.astype(BF)) for b in range(B)]

    in_maps = []
    p = np.arange(128, dtype=np.float32)
    for c in range(8):
        b = c // 2
        blocks = _core_row_blocks(c)
        rows = np.concatenate([np.arange(pos * CH, (pos + 1) * CH) for pos in blocks])
        xTq = np.ascontiguousarray(x[b][rows].T.astype(BF))
        # mstart[:, jt]: causal start for the diagonal block (slot jt//4)
        mstart = np.empty((128, 16), np.float32)
        for jt in range(16):
            ibase = blocks[jt // 4] * CH
            mstart[:, jt] = jt * 128 + p - ibase
        mstart = np.clip(mstart, -512, 512).astype(BF)
        in_maps.append(
            {
                "xT": xT[b],
                "xTq": xTq,
                "w_q": w_q,
                "w_kv": w_kv,
                "wb_out": wb,
                "mstart": mstart,
                "iota": iota,
            }
        )

    res = run_bass_kernel_spmd(nc, in_maps, list(range(8)))
    LAST_RESULT = res

    out = np.empty((B, N, DIM), dtype=np.float32)
    for c in range(8):
        b = c // 2
        r = res.results[c]["out"]
        for s, pos in enumerate(_core_row_blocks(c)):
            out[b, pos * CH:(pos + 1) * CH] = r[s * CH:(s + 1) * CH]
    return out



# revision 7
# speedup vs baseline: 1.1504x; 1.1504x over previous
"""Distributed causal MHA for TRN2 (8 NeuronCores), v5: head x batch sharding.

Core c: batch c//2, heads 8*(c%2)..+8 (4 head-pairs). Each core projects
Q/K/V for its 8 heads over all 2048 tokens, runs causal attention, and emits
a PARTIAL out-projection (contraction over its 512 features); the host sums
the two partials per batch and adds the bias. This removes the duplicated
K/V projection of the old query-sharded layout.

Per-core structure: head-pair p's K/V/Q projections are interleaved as PE
filler into the previous pair's ACT-bound attention loop (keeps the PE HAM
warm and ACT never idle). Attention runs per (query-half qh, key tile jt)
with wide score MMs [128 keys x <=1024 queries]; the causal diagonal is
handled by an identity-matmul accumulating a constant -3e8 mask tile into
PSUM (zero DVE masking work); one wide exp per (head, jt) on ACT; AV
accumulates per 512-query chunk with denominators produced by a col-tiled
ones-column matmul landing on spare PSUM partitions of a shared bank.
Normalization: DVE reciprocal rows + ones-row broadcast matmuls + one DVE
mult per 512-chunk. Out-projection for the first query half is interleaved
into the last pair's attention; second half is the tail.
"""

import sys

sys.path.insert(0, "/opt/trn_rl_repo")
import numpy as np
import ml_dtypes
import concourse.bass as bass
import concourse.mybir as mybir
import concourse.tile as tile
from concourse.vector_clock import ScopedClock
from concourse.bass_utils import run_bass_kernel_spmd

B, N, DIM = 4, 2048, 1024
HEADS, DH = 16, 64
INNER = HEADS * DH
SCALE = DH ** -0.5
NEG = -3.0e8
F32 = mybir.dt.float32
BF16 = mybir.dt.bfloat16
AF = mybir.ActivationFunctionType

# (chunk, head-in-pair) -> dens/recip partition row; chosen so each den MM's
# array col-group is disjoint from the concurrently-issued AV dims MM.
DROW = {(0, 0): 96, (0, 1): 32, (1, 0): 64, (1, 1): 0}

LAST_RESULT = None


def _drain_and_barrier_patched(self, tick_clock, wait_clock):
    nop_inst = self.nc.sync.nop(nofuse=True)
    wait_clock.add_sem_waits(nop_inst.ins, ScopedClock({None: tick_clock.global_clock}))
    si = nop_inst.ins.sync_info
    waits = list(si.on_wait or []) if si else []
    if len(waits) > 1:
        nop_inst.ins.sync_info = mybir.SyncInfo(
            on_wait=waits[:1], on_update=list(si.on_update or [])
        )
        for i in range(1, len(waits)):
            extra = self.nc.sync.nop(nofuse=True)
            extra.ins.sync_info = mybir.SyncInfo(on_wait=[waits[i]], on_update=[])
    self.nc.sync.drain()
    self.nc.all_engine_barrier()
    popped = self.nc._tile_sem_poison_stack.pop()
    assert popped is self._sem_poison
    self.nc.clear_and_free_semaphores(list(self.sems.allocated().values()))
    self.nc.all_engine_barrier()


tile.TileContext._drain_and_barrier = _drain_and_barrier_patched


def _split_multi_waits(nc):
    for f in nc.m.functions:
        for bb in f.blocks:
            insts = bb.instructions
            if not any(
                i.sync_info and i.sync_info.on_wait and len(i.sync_info.on_wait) > 1
                for i in insts
            ):
                continue
            new = []
            for inst in insts:
                si = inst.sync_info
                waits = list(si.on_wait) if si and si.on_wait else []
                if len(waits) > 1:
                    for w in waits[:-1]:
                        nop = mybir.InstNoOp(
                            name=nc.get_next_instruction_name(), ins=[], outs=[]
                        )
                        nop.engine = inst.engine
                        nop.sync_info = mybir.SyncInfo(on_wait=[w], on_update=[])
                        new.append(nop)
                    inst.sync_info = mybir.SyncInfo(
                        on_wait=[waits[-1]], on_update=list(si.on_update or [])
                    )
                new.append(inst)
            bb.instructions = new


def build_graph():
    nc = bass.Bass("TRN2", target_bir_lowering=False)

    p_xT = nc.declare_dram_parameter("xT", [DIM, N], BF16, isOutput=False)
    p_wq = nc.declare_dram_parameter("w_q", [DIM, 512], BF16, isOutput=False)
    p_wk = nc.declare_dram_parameter("w_k", [DIM, 512], BF16, isOutput=False)
    p_wv = nc.declare_dram_parameter("w_v", [DIM, 512], BF16, isOutput=False)
    p_wo = nc.declare_dram_parameter("w_o", [512, DIM], BF16, isOutput=False)
    p_msk = nc.declare_dram_parameter("maskT", [128, 128], BF16, isOutput=False)
    p_id = nc.declare_dram_parameter("ident", [128, 128], BF16, isOutput=False)
    p_out = nc.declare_dram_parameter("out", [N, DIM], F32, isOutput=True)

    with tile.TileContext(nc) as tc:
        cst = tc.alloc_tile_pool(name="cst", bufs=1)
        xtp = tc.alloc_tile_pool(name="xtp", bufs=1)
        wp = tc.alloc_tile_pool(name="wp", bufs=1)
        kqp = tc.alloc_tile_pool(name="kqp", bufs=1)
        vp = tc.alloc_tile_pool(name="vp", bufs=1)
        afp = tc.alloc_tile_pool(name="afp", bufs=1)
        ewp = tc.alloc_tile_pool(name="ewp", bufs=3)
        rcp = tc.alloc_tile_pool(name="rcp", bufs=2)
        osp = tc.alloc_tile_pool(name="osp", bufs=3)
        ps_sc = tc.alloc_tile_pool(name="ps_sc", bufs=2, space="PSUM")
        ps_av = tc.alloc_tile_pool(name="ps_av", bufs=1, space="PSUM")
        ps_dn = tc.alloc_tile_pool(name="ps_dn", bufs=1, space="PSUM")
        ps_ms = tc.alloc_tile_pool(name="ps_ms", bufs=1, space="PSUM")

        maskT = cst.tile([128, 128], BF16, tag="maskT", name="maskT")
        ident = cst.tile([128, 128], BF16, tag="ident", name="ident")
        onec = cst.tile([128, 1], BF16, tag="onec", name="onec")
        oner = cst.tile([128, 64], BF16, tag="oner", name="oner")
        wsrc = cst.tile([1, 8], F32, tag="wsrc", name="wsrc")
        wdst = cst.tile([1, 8], BF16, tag="wdst", name="wdst")

        nc.vector.memset(onec[:, :], 1.0)
        nc.vector.memset(oner[:, :], 1.0)
        nc.vector.memset(wsrc[:, :], 0.0)
        # warm up the exp table-set load while DMAs stream in
        nc.scalar.activation(wdst[:, :], wsrc[:, :], AF.Exp, scale=1.0)

        nc.sync.dma_start(maskT[:, :], p_msk[:, :])
        nc.sync.dma_start(ident[:, :], p_id[:, :])

        xt = [xtp.tile([128, N], BF16, tag=f"xt{i}", name=f"xt{i}") for i in range(8)]
        for i in range(8):
            nc.sync.dma_start(xt[i][:, :], p_xT[i * 128:(i + 1) * 128, :])
        wq = [wp.tile([128, 512], BF16, tag=f"wq{i}", name=f"wq{i}") for i in range(8)]
        wk = [wp.tile([128, 512], BF16, tag=f"wk{i}", name=f"wk{i}") for i in range(8)]
        wv = [wp.tile([128, 512], BF16, tag=f"wv{i}", name=f"wv{i}") for i in range(8)]
        for i in range(8):
            nc.sync.dma_start(wk[i][:, :], p_wk[i * 128:(i + 1) * 128, :])
            nc.sync.dma_start(wq[i][:, :], p_wq[i * 128:(i + 1) * 128, :])
            nc.sync.dma_start(wv[i][:, :], p_wv[i * 128:(i + 1) * 128, :])
        wo = [wp.tile([128, DIM], BF16, tag=f"wo{i}", name=f"wo{i}") for i in range(4)]
        for i in range(4):
            nc.sync.dma_start(wo[i][:, :], p_wo[i * 128:(i + 1) * 128, :])

        kt = [kqp.tile([128, N], BF16, tag=f"kt{p}", name=f"kt{p}") for p in range(4)]
        qt = [kqp.tile([128, N], BF16, tag=f"qt{p}", name=f"qt{p}") for p in range(4)]
        vsb = [vp.tile([128, 512], BF16, tag=f"vs{t}", name=f"vs{t}") for t in range(16)]
        af = [afp.tile([128, N], BF16, tag=f"af{p}", name=f"af{p}") for p in range(4)]

        # ---------------- projection emitters (PE filler units) ----------
        def k_chunk(p, tc4):
            def go():
                ps = ps_ms.tile([128, 512], F32, tag="mm", name=f"psk{p}_{tc4}")
                for k8 in range(8):
                    nc.tensor.matmul(
                        ps[:, :],
                        wk[k8][:, p * 128:(p + 1) * 128],
                        xt[k8][:, tc4 * 512:(tc4 + 1) * 512],
                        start=(k8 == 0),
                        stop=(k8 == 7),
                    )
                nc.vector.tensor_copy(kt[p][:, tc4 * 512:(tc4 + 1) * 512], ps[:, :])
            return go

        def q_chunk(p, tc4):
            def go():
                ps = ps_ms.tile([128, 512], F32, tag="mm", name=f"psq{p}_{tc4}")
                for k8 in range(8):
                    nc.tensor.matmul(
                        ps[:, :],
                        wq[k8][:, p * 128:(p + 1) * 128],
                        xt[k8][:, tc4 * 512:(tc4 + 1) * 512],
                        start=(k8 == 0),
                        stop=(k8 == 7),
                    )
                nc.vector.tensor_copy(qt[p][:, tc4 * 512:(tc4 + 1) * 512], ps[:, :])
            return go

        def v_chunk(p, vg):
            # two token-tiles (2vg, 2vg+1) of this pair's V, side by side in
            # one PSUM bank: first MM start marks the whole bank pending, so
            # the second token-tile's first write correctly overwrites.
            def go():
                ps = ps_ms.tile([128, 512], F32, tag="mm", name=f"psv{p}_{vg}")
                for ts in range(2):
                    tt = 2 * vg + ts
                    for k8 in range(8):
                        nc.tensor.matmul(
                            ps[:, ts * 128:(ts + 1) * 128],
                            xt[k8][:, tt * 128:(tt + 1) * 128],
                            wv[k8][:, p * 128:(p + 1) * 128],
                            start=(ts == 0 and k8 == 0),
                            stop=(ts == 1 and k8 == 7),
                        )
                for ts in range(2):
                    tt = 2 * vg + ts
                    nc.vector.tensor_copy(
                        vsb[tt][:, p * 128:(p + 1) * 128],
                        ps[:, ts * 128:(ts + 1) * 128],
                    )
            return go

        def make_proj_fillers(p):
            return [
                k_chunk(p, 0), q_chunk(p, 0), q_chunk(p, 1), v_chunk(p, 0),
                v_chunk(p, 1), k_chunk(p, 1), v_chunk(p, 2), v_chunk(p, 3),
                k_chunk(p, 2), q_chunk(p, 2), v_chunk(p, 4), v_chunk(p, 5),
                q_chunk(p, 3), k_chunk(p, 3), v_chunk(p, 6), v_chunk(p, 7),
            ]

        def p3_unit(it, oc):
            def go():
                po = ps_ms.tile([128, 512], F32, tag="mm", name=f"po{it}_{oc}")
                for p4 in range(4):
                    nc.tensor.matmul(
                        po[:, :],
                        af[p4][:, it * 128:(it + 1) * 128],
                        wo[p4][:, oc * 512:(oc + 1) * 512],
                        start=(p4 == 0),
                        stop=(p4 == 3),
                    )
                ot = osp.tile([128, 512], F32, tag="os", name=f"os{it}_{oc}")
                nc.vector.tensor_copy(ot[:, :], po[:, :])
                nc.sync.dma_start(
                    p_out[it * 128:(it + 1) * 128, oc * 512:(oc + 1) * 512],
                    ot[:, :],
                )
            return go

        # ---------------- attention for one (pair, query-half) -----------
        def attention(p, qh, pacer):
            steps = 8 if qh == 0 else 16
            av = ps_av.tile([128, 1024], F32, tag="av", name=f"av{p}_{qh}")
            dn = ps_dn.tile([128, 512], F32, tag="dn", name=f"dn{p}_{qh}")
            for jt in range(steps):
                qs = max(jt * 128, qh * 1024)
                qe = (qh + 1) * 1024
                W = qe - qs
                qoff = qs - qh * 1024
                diag = jt >= qh * 8
                for hi in (0, 1):
                    off = 64 * hi
                    sc = ps_sc.tile([128, 1024], F32, tag="sc", name=f"sc{jt}_{hi}")
                    w0 = min(512, W)
                    nc.tensor.matmul(
                        sc[:, 0:w0],
                        kt[p][off:off + 64, jt * 128:(jt + 1) * 128],
                        qt[p][off:off + 64, qs:qs + w0],
                        start=True,
                        stop=(not diag),
                    )
                    if W > 512:
                        nc.tensor.matmul(
                            sc[:, 512:W],
                            kt[p][off:off + 64, jt * 128:(jt + 1) * 128],
                            qt[p][off:off + 64, qs + 512:qe],
                            start=True,
                            stop=True,
                        )
                    if diag:
                        nc.tensor.matmul(
                            sc[:, 0:128], ident[:, :], maskT[:, :],
                            start=False, stop=True,
                        )
                    eW = ewp.tile([128, 1024], BF16, tag="ew", name=f"ew{jt}_{hi}")
                    nc.scalar.activation(eW[:, 0:W], sc[:, 0:W], AF.Exp, scale=SCALE)
                    for c in (0, 1):
                        lo = max(qoff, c * 512)
                        hi2 = (c + 1) * 512
                        if lo >= hi2:
                            continue
                        st = jt == 0
                        sp = jt == qh * 8 + 4 * c + 3
                        nc.tensor.matmul(
                            av[off:off + 64, lo:hi2],
                            vsb[jt][:, (2 * p + hi) * 64:(2 * p + hi + 1) * 64],
                            eW[:, lo - qoff:hi2 - qoff],
                            start=st,
                            stop=sp,
                        )
                        dr = DROW[(c, hi)]
                        nc.tensor.matmul(
                            dn[dr:dr + 1, lo - c * 512:hi2 - c * 512],
                            onec[:, :],
                            eW[:, lo - qoff:hi2 - qoff],
                            start=st,
                            stop=sp,
                            tile_position=(0, dr),
                        )
                pacer.step()
            # normalization: recip of den rows, ones-row broadcast, one mult
            rc = rcp.tile([128, 512], BF16, tag="rc", name=f"rc{p}_{qh}")
            with nc.allow_low_precision(reason="softmax denom reciprocal in bf16"):
                for c in (0, 1):
                    for hi in (0, 1):
                        r = DROW[(c, hi)]
                        nc.vector.reciprocal(rc[r:r + 1, :], dn[r:r + 1, :])
            for c in (0, 1):
                rb = ps_ms.tile([128, 512], F32, tag="mm", name=f"rb{p}_{qh}_{c}")
                r0 = DROW[(c, 0)]
                r1 = DROW[(c, 1)]
                nc.tensor.matmul(
                    rb[0:64, :], oner[r0:r0 + 1, :], rc[r0:r0 + 1, :],
                    start=True, stop=True, tile_position=(r0, 0),
                )
                nc.tensor.matmul(
                    rb[64:128, :], oner[r1:r1 + 1, :], rc[r1:r1 + 1, :],
                    start=True, stop=True, tile_position=(r1, 64),
                )
                rbs = rcp.tile([128, 512], F32, tag="rbs", name=f"rbs{p}_{qh}_{c}")
                nc.vector.tensor_copy(rbs[:, :], rb[:, :])
                nc.vector.tensor_mul(
                    af[p][:, qh * 1024 + c * 512:qh * 1024 + (c + 1) * 512],
                    av[:, c * 512:(c + 1) * 512],
                    rbs[:, :],
                )

        class Pacer:
            def __init__(self, fillers, total_steps):
                self.fillers = fillers
                self.total = max(1, total_steps)
                self.done = 0
                self.emitted = 0

            def step(self):
                self.done += 1
                want = (len(self.fillers) * self.done) // self.total
                while self.emitted < want:
                    self.fillers[self.emitted]()
                    self.emitted += 1

            def drain(self):
                while self.emitted < len(self.fillers):
                    self.fillers[self.emitted]()
                    self.emitted += 1

        # ---------------- main schedule ----------------------------------
        for f in make_proj_fillers(0):
            f()

        for p in range(4):
            if p < 3:
                pacer = Pacer(make_proj_fillers(p + 1), 24)
                attention(p, 0, pacer)
                attention(p, 1, pacer)
                pacer.drain()
            else:
                attention(p, 0, Pacer([], 8))
                p3q0 = Pacer([p3_unit(it, oc) for it in range(8) for oc in range(2)], 16)
                attention(p, 1, p3q0)
                p3q0.drain()

        for it in range(8, 16):
            for oc in range(2):
                p3_unit(it, oc)()

        for pool in (ps_ms, ps_dn, ps_av, ps_sc, osp, rcp, ewp, afp, vp, kqp, wp, xtp, cst):
            pool.release()

    _split_multi_waits(nc)
    return nc


_GRAPH = None


def _get_graph():
    global _GRAPH
    if _GRAPH is None:
        _GRAPH = build_graph()
    return _GRAPH


def kernel(x, mask, w_qkv, w_out, b_out):
    global LAST_RESULT
    x = np.asarray(x, dtype=np.float32)
    w_qkv = np.asarray(w_qkv, dtype=np.float32)
    w_out = np.asarray(w_out, dtype=np.float32)
    b_out = np.asarray(b_out, dtype=np.float32)

    nc = _get_graph()

    BF = ml_dtypes.bfloat16
    xT = [np.ascontiguousarray(x[b].T.astype(BF)) for b in range(B)]
    ii = np.arange(128)
    maskT = np.where(ii[None, :] >= ii[:, None], 0.0, NEG).astype(BF)
    ident = np.eye(128, dtype=np.float32).astype(BF)

    halves = []
    for h in range(2):
        o = 512 * h
        halves.append(
            {
                "w_q": np.ascontiguousarray(w_qkv[:, o:o + 512].astype(BF)),
                "w_k": np.ascontiguousarray(w_qkv[:, INNER + o:INNER + o + 512].astype(BF)),
                "w_v": np.ascontiguousarray(w_qkv[:, 2 * INNER + o:2 * INNER + o + 512].astype(BF)),
                "w_o": np.ascontiguousarray(w_out[o:o + 512, :].astype(BF)),
            }
        )

    in_maps = []
    for c in range(8):
        b = c // 2
        hv = halves[c % 2]
        in_maps.append(
            {
                "xT": xT[b],
                "w_q": hv["w_q"],
                "w_k": hv["w_k"],
                "w_v": hv["w_v"],
                "w_o": hv["w_o"],
                "maskT": maskT,
                "ident": ident,
            }
        )

    res = run_bass_kernel_spmd(nc, in_maps, list(range(8)))
    LAST_RESULT = res

    out = np.empty((B, N, DIM), dtype=np.float32)
    for b in range(B):
        out[b] = res.results[2 * b]["out"] + res.results[2 * b + 1]["out"] + b_out[None, :]
    return out


# revision 8
# speedup vs baseline: 1.7642x; 1.5335x over previous
"""Distributed causal MHA for TRN2 (8 NeuronCores), v6: head x batch sharding.

Core c: batch c//2, heads 8*(c%2)..+8 (4 head-pairs). Each core projects
Q/K/V for its 8 heads over all 2048 tokens, runs causal attention, and emits
a PARTIAL out-projection (contraction over its 512 features); the host sums
the two partials per batch and adds the bias.

v6 vs v5: denominators come from a 65th ones-column in V (free: AV matmul
cost is per-rhs-column), so the per-(head,jt) denominator matmuls are gone;
normalization uses ACT Ln -> Exp(scale=-1) (1/den = e^-ln den) instead of
the 8-cycle/elem DVE reciprocal; the odd head's af half is placed by a
SBUF->SBUF DMA partition shift; the jt loop is software-pipelined
(S_h0, exp_h0 || AV_h1(jt-1)+filler, S_h1, exp_h1 || AV_h0(jt)+filler)
with the scores tile single-buffered.
"""

import sys

sys.path.insert(0, "/opt/trn_rl_repo")
import numpy as np
import ml_dtypes
import concourse.bass as bass
import concourse.mybir as mybir
import concourse.tile as tile
from concourse.vector_clock import ScopedClock
from concourse.bass_utils import run_bass_kernel_spmd

B, N, DIM = 4, 2048, 1024
HEADS, DH = 16, 64
INNER = HEADS * DH
SCALE = DH ** -0.5
NEG = -3.0e8
F32 = mybir.dt.float32
BF16 = mybir.dt.bfloat16
AF = mybir.ActivationFunctionType

LAST_RESULT = None


def _drain_and_barrier_patched(self, tick_clock, wait_clock):
    nop_inst = self.nc.sync.nop(nofuse=True)
    wait_clock.add_sem_waits(nop_inst.ins, ScopedClock({None: tick_clock.global_clock}))
    si = nop_inst.ins.sync_info
    waits = list(si.on_wait or []) if si else []
    if len(waits) > 1:
        nop_inst.ins.sync_info = mybir.SyncInfo(
            on_wait=waits[:1], on_update=list(si.on_update or [])
        )
        for i in range(1, len(waits)):
            extra = self.nc.sync.nop(nofuse=True)
            extra.ins.sync_info = mybir.SyncInfo(on_wait=[waits[i]], on_update=[])
    self.nc.sync.drain()
    self.nc.all_engine_barrier()
    popped = self.nc._tile_sem_poison_stack.pop()
    assert popped is self._sem_poison
    self.nc.clear_and_free_semaphores(list(self.sems.allocated().values()))
    self.nc.all_engine_barrier()


tile.TileContext._drain_and_barrier = _drain_and_barrier_patched


def _split_multi_waits(nc):
    for f in nc.m.functions:
        for bb in f.blocks:
            insts = bb.instructions
            if not any(
                i.sync_info and i.sync_info.on_wait and len(i.sync_info.on_wait) > 1
                for i in insts
            ):
                continue
            new = []
            for inst in insts:
                si = inst.sync_info
                waits = list(si.on_wait) if si and si.on_wait else []
                if len(waits) > 1:
                    for w in waits[:-1]:
                        nop = mybir.InstNoOp(
                            name=nc.get_next_instruction_name(), ins=[], outs=[]
                        )
                        nop.engine = inst.engine
                        nop.sync_info = mybir.SyncInfo(on_wait=[w], on_update=[])
                        new.append(nop)
                    inst.sync_info = mybir.SyncInfo(
                        on_wait=[waits[-1]], on_update=list(si.on_update or [])
                    )
                new.append(inst)
            bb.instructions = new


def build_graph():
    nc = bass.Bass("TRN2", target_bir_lowering=False)

    p_xT = nc.declare_dram_parameter("xT", [DIM, N], BF16, isOutput=False)
    p_wq = nc.declare_dram_parameter("w_q", [DIM, 512], BF16, isOutput=False)
    p_wk = nc.declare_dram_parameter("w_k", [DIM, 512], BF16, isOutput=False)
    p_wv = nc.declare_dram_parameter("w_v", [DIM, 512], BF16, isOutput=False)
    p_wo = nc.declare_dram_parameter("w_o", [512, DIM], BF16, isOutput=False)
    p_msk = nc.declare_dram_parameter("maskT", [128, 128], BF16, isOutput=False)
    p_id = nc.declare_dram_parameter("ident", [128, 128], BF16, isOutput=False)
    p_out = nc.declare_dram_parameter("out", [N, DIM], F32, isOutput=True)

    with tile.TileContext(nc) as tc:
        cst = tc.alloc_tile_pool(name="cst", bufs=1)
        xtp = tc.alloc_tile_pool(name="xtp", bufs=1)
        wp = tc.alloc_tile_pool(name="wp", bufs=1)
        kqp = tc.alloc_tile_pool(name="kqp", bufs=1)
        vp = tc.alloc_tile_pool(name="vp", bufs=1)
        afp = tc.alloc_tile_pool(name="afp", bufs=1)
        ewp = tc.alloc_tile_pool(name="ewp", bufs=4)
        rcp = tc.alloc_tile_pool(name="rcp", bufs=2)
        osp = tc.alloc_tile_pool(name="osp", bufs=3)
        ps_sc = tc.alloc_tile_pool(name="ps_sc", bufs=1, space="PSUM")
        ps_av = tc.alloc_tile_pool(name="ps_av", bufs=2, space="PSUM")
        ps_ms = tc.alloc_tile_pool(name="ps_ms", bufs=2, space="PSUM")

        maskT = cst.tile([128, 128], BF16, tag="maskT", name="maskT")
        ident = cst.tile([128, 128], BF16, tag="ident", name="ident")
        oner = cst.tile([128, 64], BF16, tag="oner", name="oner")
        wsrc = cst.tile([1, 8], F32, tag="wsrc", name="wsrc")
        wdst = cst.tile([1, 8], F32, tag="wdst", name="wdst")

        nc.vector.memset(oner[:, :], 1.0)
        nc.vector.memset(wsrc[:, :], 1.0)
        # warm up the exp/ln table-set load while DMAs stream in
        nc.scalar.activation(wdst[:, :], wsrc[:, :], AF.Ln, scale=1.0)
        nc.scalar.activation(wdst[:, :], wsrc[:, :], AF.Exp, scale=-1.0)

        nc.sync.dma_start(maskT[:, :], p_msk[:, :])
        nc.sync.dma_start(ident[:, :], p_id[:, :])

        xt = [xtp.tile([128, N], BF16, tag=f"xt{i}", name=f"xt{i}") for i in range(8)]
        for i in range(8):
            nc.sync.dma_start(xt[i][:, :], p_xT[i * 128:(i + 1) * 128, :])
        wq = [wp.tile([128, 512], BF16, tag=f"wq{i}", name=f"wq{i}") for i in range(8)]
        wk = [wp.tile([128, 512], BF16, tag=f"wk{i}", name=f"wk{i}") for i in range(8)]
        wv = [wp.tile([128, 512], BF16, tag=f"wv{i}", name=f"wv{i}") for i in range(8)]
        for i in range(8):
            nc.sync.dma_start(wk[i][:, :], p_wk[i * 128:(i + 1) * 128, :])
            nc.sync.dma_start(wq[i][:, :], p_wq[i * 128:(i + 1) * 128, :])
            nc.sync.dma_start(wv[i][:, :], p_wv[i * 128:(i + 1) * 128, :])
        wo = [wp.tile([128, DIM], BF16, tag=f"wo{i}", name=f"wo{i}") for i in range(4)]
        for i in range(4):
            nc.sync.dma_start(wo[i][:, :], p_wo[i * 128:(i + 1) * 128, :])

        kt = [kqp.tile([128, N], BF16, tag=f"kt{p}", name=f"kt{p}") for p in range(4)]
        qt = [kqp.tile([128, N], BF16, tag=f"qt{p}", name=f"qt{p}") for p in range(4)]
        # [tokens, 8 heads x (64 V dims + ones col)]
        vsb = [vp.tile([128, 520], BF16, tag=f"vs{t}", name=f"vs{t}") for t in range(16)]
        for t in range(16):
            nc.vector.memset(
                vsb[t][:, :].rearrange("p (g d) -> p g d", g=8)[:, :, 64:65], 1.0
            )
        af = [afp.tile([128, N], BF16, tag=f"af{p}", name=f"af{p}") for p in range(4)]

        # ---------------- projection emitters (PE filler units) ----------
        def k_chunk(p, tc4):
            def go():
                ps = ps_ms.tile([128, 512], F32, tag="mm", name=f"psk{p}_{tc4}")
                for k8 in range(8):
                    nc.tensor.matmul(
                        ps[:, :],
                        wk[k8][:, p * 128:(p + 1) * 128],
                        xt[k8][:, tc4 * 512:(tc4 + 1) * 512],
                        start=(k8 == 0),
                        stop=(k8 == 7),
                    )
                nc.vector.tensor_copy(kt[p][:, tc4 * 512:(tc4 + 1) * 512], ps[:, :])
            return go

        def q_chunk(p, tc4):
            def go():
                ps = ps_ms.tile([128, 512], F32, tag="mm", name=f"psq{p}_{tc4}")
                for k8 in range(8):
                    nc.tensor.matmul(
                        ps[:, :],
                        wq[k8][:, p * 128:(p + 1) * 128],
                        xt[k8][:, tc4 * 512:(tc4 + 1) * 512],
                        start=(k8 == 0),
                        stop=(k8 == 7),
                    )
                nc.vector.tensor_copy(qt[p][:, tc4 * 512:(tc4 + 1) * 512], ps[:, :])
            return go

        def v_chunk(p, vg):
            # two token-tiles (2vg, 2vg+1) of this pair's V side by side in
            # one PSUM bank: the first MM's start marks the whole bank
            # pending, so the second region's first write overwrites.
            def go():
                ps = ps_ms.tile([128, 512], F32, tag="mm", name=f"psv{p}_{vg}")
                for ts in range(2):
                    tt = 2 * vg + ts
                    for k8 in range(8):
                        nc.tensor.matmul(
                            ps[:, ts * 128:(ts + 1) * 128],
                            xt[k8][:, tt * 128:(tt + 1) * 128],
                            wv[k8][:, p * 128:(p + 1) * 128],
                            start=(ts == 0 and k8 == 0),
                            stop=(ts == 1 and k8 == 7),
                        )
                for ts in range(2):
                    tt = 2 * vg + ts
                    dst = vsb[tt][:, 2 * p * 65:(2 * p + 2) * 65].rearrange(
                        "p (g d) -> p g d", g=2
                    )[:, :, 0:64]
                    src = ps[:, ts * 128:(ts + 1) * 128].rearrange(
                        "p (g d) -> p g d", g=2
                    )
                    nc.vector.tensor_copy(dst, src)
            return go

        def make_proj_fillers(p):
            return [
                k_chunk(p, 0), q_chunk(p, 0), q_chunk(p, 1), v_chunk(p, 0),
                v_chunk(p, 1), k_chunk(p, 1), v_chunk(p, 2), v_chunk(p, 3),
                k_chunk(p, 2), q_chunk(p, 2), v_chunk(p, 4), v_chunk(p, 5),
                q_chunk(p, 3), k_chunk(p, 3), v_chunk(p, 6), v_chunk(p, 7),
            ]

        def p3_unit(it, oc):
            def go():
                po = ps_ms.tile([128, 512], F32, tag="mm", name=f"po{it}_{oc}")
                for p4 in range(4):
                    nc.tensor.matmul(
                        po[:, :],
                        af[p4][:, it * 128:(it + 1) * 128],
                        wo[p4][:, oc * 512:(oc + 1) * 512],
                        start=(p4 == 0),
                        stop=(p4 == 3),
                    )
                ot = osp.tile([128, 512], F32, tag="os", name=f"os{it}_{oc}")
                nc.vector.tensor_copy(ot[:, :], po[:, :])
                nc.sync.dma_start(
                    p_out[it * 128:(it + 1) * 128, oc * 512:(oc + 1) * 512],
                    ot[:, :],
                )
            return go

        # ---------------- attention for one (pair, query-half) -----------
        def attention(p, qh, pacer):
            steps = 8 if qh == 0 else 16
            av = [
                ps_av.tile([65, 1024], F32, tag="av", name=f"av{p}_{qh}_{hi}")
                for hi in (0, 1)
            ]

            def scores(jt, hi):
                qs = max(jt * 128, qh * 1024)
                qe = (qh + 1) * 1024
                W = qe - qs
                diag = jt >= qh * 8
                off = 64 * hi
                sc = ps_sc.tile([128, 1024], F32, tag="sc", name=f"sc{jt}_{hi}")
                w0 = min(512, W)
                nc.tensor.matmul(
                    sc[:, 0:w0],
                    kt[p][off:off + 64, jt * 128:(jt + 1) * 128],
                    qt[p][off:off + 64, qs:qs + w0],
                    start=True,
                    stop=(not diag),
                )
                if W > 512:
                    nc.tensor.matmul(
                        sc[:, 512:W],
                        kt[p][off:off + 64, jt * 128:(jt + 1) * 128],
                        qt[p][off:off + 64, qs + 512:qe],
                        start=True,
                        stop=True,
                    )
                if diag:
                    nc.tensor.matmul(
                        sc[:, 0:128], ident[:, :], maskT[:, :],
                        start=False, stop=True,
                    )
                eW = ewp.tile([128, 1024], BF16, tag="ew", name=f"ew{jt}_{hi}")
                nc.scalar.activation(eW[:, 0:W], sc[:, 0:W], AF.Exp, scale=SCALE)
                return eW

            def av_accum(jt, hi, eW):
                qs = max(jt * 128, qh * 1024)
                qoff = qs - qh * 1024
                h = 2 * p + hi
                for c in (0, 1):
                    lo = max(qoff, c * 512)
                    hi2 = (c + 1) * 512
                    if lo >= hi2:
                        continue
                    nc.tensor.matmul(
                        av[hi][:, lo:hi2],
                        vsb[jt][:, h * 65:(h + 1) * 65],
                        eW[:, lo - qoff:hi2 - qoff],
                        start=(jt == 0),
                        stop=(jt == qh * 8 + 4 * c + 3),
                    )

            prev = {0: None, 1: None}
            for jt in range(steps):
                ew0 = scores(jt, 0)
                if prev[1] is not None:
                    av_accum(jt - 1, 1, prev[1])
                pacer.step()
                ew1 = scores(jt, 1)
                av_accum(jt, 0, ew0)
                prev[1] = ew1
                pacer.step()
            av_accum(steps - 1, 1, prev[1])

            # normalization: 1/den = exp(-ln(den)) on ACT, broadcast via
            # ones-row matmul, one DVE mult per chunk. Odd head's af half is
            # partition-shifted into place by an SBUF->SBUF DMA.
            af1t = rcp.tile([64, 1024], BF16, tag="af1", name=f"af1_{p}_{qh}")
            for hi in (0, 1):
                lnb = rcp.tile([65, 1024], F32, tag="lnb", name=f"lnb{p}_{qh}_{hi}")
                rec = rcp.tile([65, 1024], BF16, tag="rec", name=f"rec{p}_{qh}_{hi}")
                nc.scalar.activation(
                    lnb[64:65, :], av[hi][64:65, 0:1024], AF.Ln, scale=1.0
                )
                nc.scalar.activation(
                    rec[64:65, :], lnb[64:65, :], AF.Exp, scale=-1.0
                )
                for c in (0, 1):
                    rb = ps_ms.tile([128, 512], F32, tag="mm", name=f"rb{p}_{qh}_{hi}_{c}")
                    nc.tensor.matmul(
                        rb[0:64, :],
                        oner[64:65, :],
                        rec[64:65, c * 512:(c + 1) * 512],
                        start=True,
                        stop=True,
                    )
                    rbs = rcp.tile([64, 512], F32, tag="rbs", name=f"rbs{p}_{qh}_{hi}_{c}")
                    nc.vector.tensor_copy(rbs[:, :], rb[0:64, :])
                    dst = (
                        af[p][0:64, qh * 1024 + c * 512:qh * 1024 + (c + 1) * 512]
                        if hi == 0
                        else af1t[:, c * 512:(c + 1) * 512]
                    )
                    nc.vector.tensor_mul(
                        dst, av[hi][0:64, c * 512:(c + 1) * 512], rbs[:, :]
                    )
            nc.sync.dma_start(
                af[p][64:128, qh * 1024:(qh + 1) * 1024], af1t[:, :]
            )

        class Pacer:
            def __init__(self, fillers, total_steps):
                self.fillers = fillers
                self.total = max(1, total_steps)
                self.done = 0
                self.emitted = 0

            def step(self):
                self.done += 1
                want = (len(self.fillers) * self.done) // self.total
                while self.emitted < want:
                    self.fillers[self.emitted]()
                    self.emitted += 1

            def drain(self):
                while self.emitted < len(self.fillers):
                    self.fillers[self.emitted]()
                    self.emitted += 1

        # ---------------- main schedule ----------------------------------
        for f in make_proj_fillers(0):
            f()

        for p in range(4):
            if p < 3:
                pacer = Pacer(make_proj_fillers(p + 1), 48)
                attention(p, 0, pacer)
                attention(p, 1, pacer)
                pacer.drain()
            else:
                attention(p, 0, Pacer([], 16))
                p3q0 = Pacer(
                    [p3_unit(it, oc) for it in range(8) for oc in range(2)], 32
                )
                attention(p, 1, p3q0)
                p3q0.drain()

        for it in range(8, 16):
            for oc in range(2):
                p3_unit(it, oc)()

        for pool in (ps_ms, ps_av, ps_sc, osp, rcp, ewp, afp, vp, kqp, wp, xtp, cst):
            pool.release()

    _split_multi_waits(nc)
    return nc


_GRAPH = None


def _get_graph():
    global _GRAPH
    if _GRAPH is None:
        _GRAPH = build_graph()
    return _GRAPH


def kernel(x, mask, w_qkv, w_out, b_out):
    global LAST_RESULT
    x = np.asarray(x, dtype=np.float32)
    w_qkv = np.asarray(w_qkv, dtype=np.float32)
    w_out = np.asarray(w_out, dtype=np.float32)
    b_out = np.asarray(b_out, dtype=np.float32)

    nc = _get_graph()

    BF = ml_dtypes.bfloat16
    xT = [np.ascontiguousarray(x[b].T.astype(BF)) for b in range(B)]
    ii = np.arange(128)
    maskT = np.where(ii[None, :] >= ii[:, None], 0.0, NEG).astype(BF)
    ident = np.eye(128, dtype=np.float32).astype(BF)

    halves = []
    for h in range(2):
        o = 512 * h
        halves.append(
            {
                "w_q": np.ascontiguousarray(w_qkv[:, o:o + 512].astype(BF)),
                "w_k": np.ascontiguousarray(w_qkv[:, INNER + o:INNER + o + 512].astype(BF)),
                "w_v": np.ascontiguousarray(w_qkv[:, 2 * INNER + o:2 * INNER + o + 512].astype(BF)),
                "w_o": np.ascontiguousarray(w_out[o:o + 512, :].astype(BF)),
            }
        )

    in_maps = []
    for c in range(8):
        b = c // 2
        hv = halves[c % 2]
        in_maps.append(
            {
                "xT": xT[b],
                "w_q": hv["w_q"],
                "w_k": hv["w_k"],
                "w_v": hv["w_v"],
                "w_o": hv["w_o"],
                "maskT": maskT,
                "ident": ident,
            }
        )

    res = run_bass_kernel_spmd(nc, in_maps, list(range(8)))
    LAST_RESULT = res

    out = np.empty((B, N, DIM), dtype=np.float32)
    for b in range(B):
        out[b] = res.results[2 * b]["out"] + res.results[2 * b + 1]["out"] + b_out[None, :]
    return out
